# revision 32
# baseline (speedup 1.0000x reference)
"""Batched attention with K/V projection on 8 TRN2 NeuronCores.

reference (per batch b):
    keys   = states @ Wk + bk                  [S_kv, H]
    values = states @ Wv + bv                  [S_kv, H]
    scores = (query @ keys.T) / sqrt(H)        [S_q, S_kv]
    attn   = softmax(mask * scores, axis=-1)
    out    = attn @ values                     [S_q, H]

Sharding: pure data parallel — batch b -> core b (B == n_cores == 8).

Layout: every tensor staged host-side with its contraction dim leading:
    queryT [H, S_q] (pre-scaled by 1/sqrt(H)), statesT [DIN, S_kv],
    maskT [S_kv, S_q].

Final design. The kernel is PE-bound: the four real matmul groups
(K-proj 65536 + V-proj 65536 + scores 131072 + PV 131072 PE cycles =
393k cycles = 163.8 us @2.4GHz) are packed back-to-back with zero PE
idle between the first DMA landing (~2.4 us) and the last PV (sim
169.65 us total vs 188.9 us for the prior ones-matmul design):
  - Mask is uint8 (exp computes exp(m*s/256) via the activation's
    scalar scale; abs err 1/512). Input traffic 12.2 MB; output bf16.
  - Softmax denominator is OFF the PE (the old design spent 33k PE
    cycles = 13.7 us on accumulating ones-matmuls): exp tiles land in a
    contiguous [128, 16, 512] bf16 tile per q-tile; for qt 0-2 one DVE
    reduce_sum (strided AP, innermost = kv-chunk) runs inside the PV
    window; for the last qt (where 1/S is needed right after the last
    exp) per-kvc tensor_adds on the otherwise-idle Pool engine track
    the exp stream. Then partition_all_reduce (Pool) + full-tile
    reciprocal (DVE) give the [128,512] broadcast 1/S directly.
  - Scores issue 2 blocks ahead of the DVE mask-mult ACROSS q-tile
    boundaries, so the PE flows through qt transitions with no stall
    (was ~300-700 ns per transition).
  - DMA: statesT split into front/back column-halves interleaved over
    the SP and ACT HWDGE queues to match the K-wave's per-dc consume
    rate; masks right after the st-odd halves on ACT; qT on SP in
    column-halves. Measured regressions (prev session): masks on SP,
    masks after the K waves, masks half on SWDGE.
  - PSUM: 8 banks via tags a3/b4/c1 (projection waves use all 8;
    attention: scores rotate a, PV holds b, c idles until the final
    half-tile). Last q-tile PV is hc-outer; its final hc splits into
    column-halves on separate banks (b/c) with stores on the idle SP
    HWDGE queue, trimming the post-PE drain to ~3 us.
"""

import os
import contextlib
import numpy as np
import ml_dtypes

B, SQ, SKV, DIN, H = 8, 2048, 2048, 1024, 512
P = 128
HC = H // P      # 4  h-chunks of 128
DC = DIN // P    # 8  d-chunks of 128
KVC = SKV // P   # 16 kv-chunks of 128
QT = SQ // 512   # 4  q-tiles of 512
ST = SKV // 512  # 4  s-tiles of 512

LAST_EXEC_NS = None
LAST_RESULTS = None
_NC = None


def _build(repeat=1):
    import concourse.bacc as bacc
    import concourse.tile as tile
    import concourse.mybir as mybir
    from concourse import bass_isa

    f32 = mybir.dt.float32
    bf16 = mybir.dt.bfloat16
    Exp = mybir.ActivationFunctionType.Exp
    Ident = mybir.ActivationFunctionType.Identity

    nc = bacc.Bacc("TRN2", target_bir_lowering=False, debug=False, num_devices=8, num_swdge_queues=4)
    qT_d = nc.dram_tensor("qT", [H, SQ], bf16, kind="ExternalInput").ap()
    sT_d = nc.dram_tensor("sT", [DIN, SKV], bf16, kind="ExternalInput").ap()
    mT_d = nc.dram_tensor("mT", [SKV, SQ], mybir.dt.uint8, kind="ExternalInput").ap()
    wk_d = nc.dram_tensor("wk", [DIN, H], bf16, kind="ExternalInput").ap()
    wv_d = nc.dram_tensor("wv", [DIN, H], bf16, kind="ExternalInput").ap()
    bk_d = nc.dram_tensor("bk", [H], f32, kind="ExternalInput").ap()
    bv_d = nc.dram_tensor("bv", [H], f32, kind="ExternalInput").ap()
    out_d = nc.dram_tensor("out", [H, SQ], bf16, kind="ExternalOutput").ap()

    with tile.TileContext(nc) as tc:
        with tc.tile_pool(name="const", bufs=1) as cpool, \
             tc.tile_pool(name="big", bufs=1) as big, \
             tc.tile_pool(name="ebig", bufs=2) as ebpool, \
             tc.tile_pool(name="spool", bufs=2) as spool, \
             tc.tile_pool(name="tmp", bufs=4) as tpool, \
             tc.tile_pool(name="osb", bufs=4) as opool, \
             tc.tile_pool(name="ivb", bufs=2) as ipool, \
             tc.tile_pool(name="ps", bufs=1, space="PSUM") as psp, \
             (tc.For_i(0, repeat, 1, hint_engines=(
                  mybir.EngineType.PE, mybir.EngineType.DVE,
                  mybir.EngineType.Activation, mybir.EngineType.Pool,
                  mybir.EngineType.SP))
              if repeat > 1 else contextlib.nullcontext()):

            # resident inputs (bf16); statesT + Wk first — they gate the PE.
            # statesT chunks split into front (cols 0:1024, K-wave-1 fuel)
            # and back halves, interleaved across the SP and ACT HWDGE
            # queues so the supply rate matches the K-wave's per-dc
            # consumption rate (the SP queue alone falls ~0.3us/dc behind).
            wk_sb = big.tile([P, DC, H], bf16)
            wv_sb = big.tile([P, DC, H], bf16)
            st_sb = big.tile([P, DC, SKV], bf16)
            qT_sb = big.tile([P, HC, SQ], bf16)
            m_sb = big.tile([P, KVC, SQ], mybir.dt.uint8)
            HF = SKV // 2

            def st_half(dc, back, eng):
                sl = slice(HF, SKV) if back else slice(0, HF)
                eng.dma_start(st_sb[:, dc, sl], sT_d[dc * P:(dc + 1) * P, sl])

            # first chunk's front half in two 512-col pieces: the very
            # first matmul needs only cols 0:512, saving ~0.4us of startup.
            nc.scalar.dma_start(st_sb[:, 0, 0:512], sT_d[0:P, 0:512])
            nc.scalar.dma_start(st_sb[:, 0, 512:HF], sT_d[0:P, 512:HF])
            nc.sync.dma_start(wk_sb[:, 0], wk_d[0:P])
            for dc in range(1, DC):
                nc.sync.dma_start(wk_sb[:, dc], wk_d[dc * P:(dc + 1) * P])
                st_half(dc, False, nc.scalar if dc % 2 else nc.sync)
            for dc in range(1, DC, 2):
                st_half(dc, True, nc.scalar)

            # PE warm-up: an 8-cycle matmul on a memset tile right after
            # the opening barrier, so the PE's power-state ramp (0.65 ->
            # 2.4 GHz over ~3us of activity) starts ticking ~2us before
            # the first DMA-gated real matmul lands. Costs ~10ns; on HW
            # that ramps from first compute (not the barrier) it upgrades
            # the first real matmuls from half speed.
            wrm = cpool.tile([P, 8], bf16)
            nc.gpsimd.memset(wrm, 1.0)
            wps = psp.tile([8, 8], f32, tag="a", bufs=3, name="warm")
            nc.tensor.matmul(wps, wrm, wrm, start=True, stop=True)

            # constants (tiny; after the PE-gating loads in queue order)
            bk_sb = cpool.tile([P, HC], f32)
            nc.sync.dma_start(bk_sb, bk_d.rearrange("(c p) -> p c", p=P))
            bv_row = cpool.tile([1, H], f32)
            nc.sync.dma_start(bv_row, bv_d.rearrange("(o h) -> o h", o=1))
            bv_bc = cpool.tile([P, H], f32)
            nc.gpsimd.partition_broadcast(bv_bc, bv_row)

            # qT on the SP queue in column-halves (front cols feed the
            # early softmax; back cols aren't read until qt2). Masks go
            # on the ACT queue right after the st-odd halves — earlier
            # than the old qT-then-masks order, which also helps the
            # repeat-loop steady state (mask reloads aren't gated behind
            # the late qT_sb release). Measured regressions (prev
            # session): masks on SP, masks after the K waves, masks half
            # on SWDGE.
            for kvc in range(KVC):
                nc.scalar.dma_start(m_sb[:, kvc], mT_d[kvc * P:(kvc + 1) * P])
            # st even-backs BEFORE qT on SP: K-wave-2 consumes the backs at
            # ~16us while qT isn't read until the early-scores (~30us), so
            # this order is robust even if the HW queue only sustains
            # ~118 GB/s (the sim's DMA model is ~3x faster; under it both
            # orders are slack).
            for dc in range(0, DC, 2):
                st_half(dc, True, nc.sync)
            for hc in range(HC):
                nc.sync.dma_start(qT_sb[:, hc, 0:HF], qT_d[hc * P:(hc + 1) * P, 0:HF])
            for hc in range(HC):
                nc.sync.dma_start(qT_sb[:, hc, HF:SQ], qT_d[hc * P:(hc + 1) * P, HF:SQ])
            for dc in range(DC):
                nc.sync.dma_start(wv_sb[:, dc], wv_d[dc * P:(dc + 1) * P])

            kT_sb = big.tile([P, HC, SKV], bf16)
            v_sb = big.tile([P, KVC, H], bf16)

            # PSUM budget: 8 banks shared across phases via tags —
            #   "a" x3: projection-wave psums 0-2, then scores rotation
            #   "b" x4: projection-wave psums 3-6, then PV accumulators
            #   "c" x1: projection-wave psum 7, then the S accumulator
            def wave_tile(i, nm):
                tag = "a" if i < 3 else ("b" if i < 7 else "c")
                return psp.tile([P, 512], f32, tag=tag,
                                bufs=(3 if i < 3 else 4 if i < 7 else 1), name=nm)

            # projections: dc-outer waves of 8 PSUM banks. kT drains split
            # ACT (Identity+bias) / DVE (tensor_scalar_add); v drains DVE.
            kjobs = [(hc, st) for st in range(ST) for hc in range(HC)]
            for w, wave in enumerate((kjobs[:8], kjobs[8:])):
                psums = [wave_tile(i, f"pj{w}_{i}") for i in range(8)]
                for dc in range(DC):
                    for (hc, st), kp in zip(wave, psums):
                        nc.tensor.matmul(kp, wk_sb[:, dc, hc * P:(hc + 1) * P],
                                         st_sb[:, dc, st * 512:(st + 1) * 512],
                                         start=(dc == 0), stop=(dc == DC - 1))
                for j, ((hc, st), kp) in enumerate(zip(wave, psums)):
                    dst = kT_sb[:, hc, st * 512:(st + 1) * 512]
                    if j % 2 == 0:
                        nc.scalar.activation(dst, kp, Ident, bias=bk_sb[:, hc:hc + 1])
                    else:
                        nc.vector.tensor_scalar_add(dst, kp, bk_sb[:, hc:hc + 1])
                del psums
            # per-qt contiguous exp tiles [P, KVC, 512]; the Pool engine
            # reduces them for the softmax denominator (off the PE).
            e_bigs = {}

            def get_ebig(qt):
                if qt not in e_bigs:
                    e_bigs[qt] = ebpool.tile([P, KVC, 512], bf16, tag="e",
                                             name=f"ebig{qt}")
                return e_bigs[qt]

            # early softmax head-start: qt0's scores/mask-mult/exp for
            # kv-chunks 0-7 need only the K-wave-1 output, qT and the
            # first 8 mask blocks — all resident mid-projection. This
            # moves ~10us of DVE/ACT work (and qt0's mask wait) into the
            # projection window.
            early = set()
            eb0 = get_ebig(0)
            for kvc in range(8):
                esp = psp.tile([P, 512], f32, tag="a", bufs=3, name=f"esp{kvc}")
                for hc in range(HC):
                    nc.tensor.matmul(esp, kT_sb[:, hc, kvc * P:(kvc + 1) * P],
                                     qT_sb[:, hc, 0:512],
                                     start=(hc == 0), stop=(hc == HC - 1))
                etm = tpool.tile([P, 512], bf16, tag="tmp", name=f"etm{kvc}")
                nc.vector.tensor_mul(etm, esp, m_sb[:, kvc, 0:512])
                nc.scalar.activation(eb0[:, kvc, :], etm, Exp, scale=1.0 / 256.0)
                early.add(kvc)

            for w, wave in enumerate((range(0, 8), range(8, 16))):
                psums = [wave_tile(i, f"pv{w}_{i}") for i in range(8)]
                for dc in range(DC):
                    for kvc, vp in zip(wave, psums):
                        nc.tensor.matmul(vp, st_sb[:, dc, kvc * P:(kvc + 1) * P],
                                         wv_sb[:, dc],
                                         start=(dc == 0), stop=(dc == DC - 1))
                for kvc, vp in zip(wave, psums):
                    nc.vector.tensor_add(v_sb[:, kvc], vp, bv_bc)
                del psums

            # attention: bulk per-q-tile phases. scores psums rotate 3
            # banks at the DVE mult's pace; the 64 PV matmuls run as one
            # unobstructed block. Softmax denominator is off the PE:
            # per-partition partial sums over the 16 exp tiles (one DVE
            # reduce during the PV window for qt 0-2; per-kvc Pool adds
            # for the last qt, where invb is needed right after the
            # scores phase), then partition_all_reduce on Pool.
            # global scores-issue queue: each (qt, kvc) scores block is
            # issued 2 ahead of its mask-mult, ACROSS qt boundaries — the
            # first two scores of qt+1 are issued before qt's PV block, so
            # the PE flows through qt transitions without a stall.
            sps = {}

            def scores(qt, kvc):
                qsl = slice(qt * 512, (qt + 1) * 512)
                sp = psp.tile([P, 512], f32, tag="a", bufs=3, name=f"sp{qt}_{kvc}")
                for hc in range(HC):
                    nc.tensor.matmul(sp, kT_sb[:, hc, kvc * P:(kvc + 1) * P],
                                     qT_sb[:, hc, qsl],
                                     start=(hc == 0), stop=(hc == HC - 1))
                sps[(qt, kvc)] = sp

            issue_q = [(qt, kvc) for qt in range(QT)
                       for kvc in range(KVC) if not (qt == 0 and kvc in early)]
            iptr = 0

            def issue_until(n):
                nonlocal iptr
                while iptr < min(n, len(issue_q)):
                    scores(*issue_q[iptr])
                    iptr += 1

            consumed = 0
            for qt in range(QT):
                qsl = slice(qt * 512, (qt + 1) * 512)
                last = qt == QT - 1
                e_big = get_ebig(qt)

                o_psums = [psp.tile([P, 512], f32, tag="b", bufs=4, name=f"op{qt}_{hc}")
                           for hc in range(HC)]
                sq = spool.tile([P, 512], f32, tag="sq", name=f"sq{qt}")

                pre = early if qt == 0 else set()
                for kvc in range(KVC):
                    if kvc not in pre:
                        issue_until(consumed + 3)
                        consumed += 1
                        tmp = tpool.tile([P, 512], bf16, tag="tmp", name=f"tm{qt}_{kvc}")
                        nc.vector.tensor_mul(tmp, sps.pop((qt, kvc)), m_sb[:, kvc, qsl])
                        nc.scalar.activation(e_big[:, kvc, :], tmp, Exp,
                                             scale=1.0 / 256.0)
                    if last:
                        if kvc == 1:
                            nc.gpsimd.tensor_add(sq, e_big[:, 0, :], e_big[:, 1, :])
                        elif kvc > 1:
                            nc.gpsimd.tensor_add(sq, sq, e_big[:, kvc, :])
                if not last:
                    nc.vector.reduce_sum(sq, e_big.rearrange("p t q -> p q t"),
                                         axis=mybir.AxisListType.X)
                sall = ipool.tile([P, 512], f32, tag="sall", name=f"sa{qt}")
                nc.gpsimd.partition_all_reduce(sall, sq, channels=P,
                                               reduce_op=bass_isa.ReduceOp.add)
                invb = ipool.tile([P, 512], f32, tag="invb", name=f"ib{qt}")
                nc.vector.reciprocal(invb, sall)

                if last:
                    # hc-outer PV on the final q-tile: each accumulator
                    # finishes a quarter of the block early, so normalize +
                    # store pipeline into the PV tail. The very last hc is
                    # split into column-halves with stores on the (idle by
                    # now) SP HWDGE queue, shortening the post-PE drain.
                    for hc in range(HC - 1):
                        for kvc in range(KVC):
                            nc.tensor.matmul(o_psums[hc],
                                             v_sb[:, kvc, hc * P:(hc + 1) * P],
                                             e_big[:, kvc, :],
                                             start=(kvc == 0), stop=(kvc == KVC - 1))
                        ot = opool.tile([P, 512], bf16, tag="o", name=f"ot{qt}_{hc}")
                        nc.vector.tensor_mul(ot, o_psums[hc], invb)
                        nc.gpsimd.dma_start(out_d[hc * P:(hc + 1) * P, qsl], ot)
                    hc = HC - 1
                    # final hc in shrinking pieces (256/128/128) on separate
                    # PSUM banks ("b" slot, then the attention-idle "c" and
                    # "a" slots) so each piece's accumulation is independent
                    # and the post-PE drain is just one [128,128] norm+store.
                    oh1 = psp.tile([P, 128], f32, tag="c", bufs=1, name="oh1")
                    oh2 = psp.tile([P, 128], f32, tag="a", bufs=3, name="oh2")
                    pieces = [(slice(0, 256), None), (slice(256, 384), oh1),
                              (slice(384, 512), oh2)]
                    for pi, (esl, opx) in enumerate(pieces):
                        hsl = slice(qt * 512 + esl.start, qt * 512 + esl.stop)
                        op = o_psums[hc][:, esl] if opx is None else opx
                        for kvc in range(KVC):
                            nc.tensor.matmul(op,
                                             v_sb[:, kvc, hc * P:(hc + 1) * P],
                                             e_big[:, kvc, esl],
                                             start=(kvc == 0), stop=(kvc == KVC - 1))
                        ot = opool.tile([P, 256], bf16, tag="o2", name=f"oh{pi}")
                        w = esl.stop - esl.start
                        nc.vector.tensor_mul(ot[:, 0:w], op, invb[:, esl])
                        nc.sync.dma_start(out_d[hc * P:(hc + 1) * P, hsl], ot[:, 0:w])
                else:
                    for kvc in range(KVC):
                        for hc in range(HC):
                            nc.tensor.matmul(o_psums[hc],
                                             v_sb[:, kvc, hc * P:(hc + 1) * P],
                                             e_big[:, kvc, :],
                                             start=(kvc == 0), stop=(kvc == KVC - 1))
                    for hc in range(HC):
                        ot = opool.tile([P, 512], bf16, tag="o", name=f"ot{qt}_{hc}")
                        nc.vector.tensor_mul(ot, o_psums[hc], invb)
                        nc.gpsimd.dma_start(out_d[hc * P:(hc + 1) * P, qsl], ot)

    nc.compile()
    return nc


def prep_in_maps(query, states, mask, Wk, bk, Wv, bv):
    query = np.asarray(query)
    states = np.asarray(states)
    mask = np.asarray(mask)
    Wk, bk, Wv, bv = (np.asarray(x) for x in (Wk, bk, Wv, bv))
    bf = ml_dtypes.bfloat16
    scale = 1.0 / np.sqrt(np.float32(H))
    wk_b = np.ascontiguousarray(Wk.astype(bf))
    wv_b = np.ascontiguousarray(Wv.astype(bf))
    bk_f = np.ascontiguousarray(bk.astype(np.float32))
    bv_f = np.ascontiguousarray(bv.astype(np.float32))
    in_maps = []
    for b in range(B):
        in_maps.append({
            "qT": np.ascontiguousarray((query[b].T * scale).astype(bf)),
            "sT": np.ascontiguousarray(states[b].T.astype(bf)),
            "mT": np.ascontiguousarray(np.clip(np.round(mask[b].T * 256.0), 0, 255).astype(np.uint8)),
            "wk": wk_b, "wv": wv_b, "bk": bk_f, "bv": bv_f,
        })
    return in_maps


def kernel(query, states, mask, Wk, bk, Wv, bv):
    global LAST_EXEC_NS, LAST_RESULTS, _NC
    from concourse.bass_utils import run_bass_kernel_spmd

    if _NC is None:
        _NC = _build()

    in_maps = prep_in_maps(query, states, mask, Wk, bk, Wv, bv)

    trace = os.environ.get("BASS_KERNEL_TRACE", "0") not in ("", "0", "false")
    try:
        res = run_bass_kernel_spmd(_NC, in_maps, core_ids=list(range(B)), trace=trace)
    except ModuleNotFoundError:
        # NTFF profile hook unavailable in this environment; rerun untraced.
        os.environ["BASS_NEVER_TRACE"] = "1"
        res = run_bass_kernel_spmd(_NC, in_maps, core_ids=list(range(B)))
    LAST_EXEC_NS = res.exec_time_ns
    LAST_RESULTS = res
    out = np.stack([res.results[b]["out"].T for b in range(B)])
    return np.ascontiguousarray(out.astype(np.float32))

